# revision 30
# baseline (speedup 1.0000x reference)
"""BiDAF attention-flow layer on 8 Trainium2 NeuronCores.

Data-parallel over batch: each core processes B/8 = 8 batches.

Math (per batch b):
  s[t,j] = h[t]·w_h + u[j]·w_u + (h[t]*w_hu)·u[j] + const
  a      = softmax_j(s)            -> only needs  sj = shu + su  (row consts cancel)
  c2q    = a @ u
  bt     = softmax_t(max_j s)      -> needs  m + sh  where m = max_j(sj)
  q2c    = bt @ h
  g      = [h | c2q | h*c2q | h*q2c]

The rank-1 bias terms b_h/b_u/b_hu shift every s[t,j] equally and cancel in
both softmaxes, so they are accepted but unused.

Layout per batch (core-local):
  htile [128, 7*200]   h rows chunked by 128 (chunk c at cols c*200..)
  hT    [100, 2*800]   h transposed (PE transpose pairs, one fused copy)
  s_ps  [128, 51]      cols 0:50 = shu+su (su via K=1 ones-matmul), col 50 = sh
  softmax on free dim; p transposed back (PE) for the c2q matmul;
  y[1,200] = sum_t e_t h_t accumulated in PSUM over chunks; q2c = y/sum(e);
  g cols 0:600 stream out as chunk-pair DMAs, cols 600:800 (needs q2c) as
  two per-batch DMAs. Engines balanced: ACT exp/copies, DVE softmax+scales,
  GPSIMD h-copy + h*c2q, PE matmuls/transposes; all DMAs on the SP HWDGE ring.
"""
import sys

if '/opt/trn_rl_repo' not in sys.path:
    sys.path.insert(0, '/opt/trn_rl_repo')

import numpy as np

B, T, J, D = 64, 800, 50, 200
NCORES = 8
BC = B // NCORES            # batches per core
P = 128
TCHUNKS = [(c * P, min(P, T - c * P)) for c in range((T + P - 1) // P)]
KCHUNKS = [(0, 100), (100, 100)]
NPAD = 256

_cache = {}
F32R = False  # f32r c2q is ~8us faster but 50x less accurate; keep exact


def _split_multi_waits(nc, max_waits=1):
    """This walrus build accepts at most one sync-wait per instruction.
    For any instruction carrying more, move the extra waits onto pure-wait
    EventSemaphore carriers inserted just before it on the same engine —
    the sequencer dispatches in order, so the blocking behavior is
    identical."""
    from concourse import mybir
    import bass_rust
    n = 0
    for f in nc.m.functions:
        for blk in f.blocks:
            insts = blk.instructions
            i = 0
            while i < len(insts):
                inst = insts[i]
                si = inst.sync_info
                if si is not None and len(si.on_wait) > max_waits:
                    waits = list(si.on_wait)
                    keep = waits[-max_waits:]
                    new = []
                    for w in waits[:-max_waits]:
                        d = mybir.InstEventSemaphore(
                            name=f"{inst.name}-sw{n}", ins=[], outs=[])
                        n += 1
                        d.engine = inst.engine
                        d.sync_info = bass_rust.SyncInfo(on_wait=[w], on_update=[])
                        new.append(d)
                    inst.sync_info = bass_rust.SyncInfo(
                        on_wait=keep, on_update=list(si.on_update))
                    for j, d in enumerate(new):
                        insts.insert(i + j, d)
                    i += len(new)
                i += 1
    return n


def _build(reps=1):
    import concourse.bass as bass
    import concourse.tile as tile
    from concourse import mybir, masks
    from contextlib import ExitStack

    f32 = mybir.dt.float32
    f32r = mybir.dt.float32r
    AF = mybir.ActivationFunctionType
    AX = mybir.AxisListType

    nc = bass.Bass()
    h_in = nc.declare_dram_parameter("h", [BC, T, D], f32, isOutput=False)
    u_in = nc.declare_dram_parameter("u", [BC, J, D], f32, isOutput=False)
    wh_in = nc.declare_dram_parameter("w_h", [D], f32, isOutput=False)
    wu_in = nc.declare_dram_parameter("w_u", [D], f32, isOutput=False)
    whu_in = nc.declare_dram_parameter("w_hu", [D], f32, isOutput=False)
    g_out = nc.declare_dram_parameter("g", [BC, T, 4 * D], f32, isOutput=True)

    with tile.TileContext(nc) as tc, ExitStack() as ctx:
        singles = ctx.enter_context(tc.tile_pool(name="singles", bufs=1))
        hpool = ctx.enter_context(tc.tile_pool(name="hpool", bufs=4))
        hTpool = ctx.enter_context(tc.tile_pool(name="hTpool", bufs=3))
        upool = ctx.enter_context(tc.tile_pool(name="upool", bufs=3))
        bsmall = ctx.enter_context(tc.tile_pool(name="bsmall", bufs=4))
        csmall = ctx.enter_context(tc.tile_pool(name="csmall", bufs=6))
        gpool = ctx.enter_context(tc.tile_pool(name="gpool", bufs=8))
        hqpool = ctx.enter_context(tc.tile_pool(name="hqpool", bufs=3))
        ps_s = ctx.enter_context(
            tc.tile_pool(name="ps_s", bufs=2, space=bass.MemorySpace.PSUM))
        ps_tp = ctx.enter_context(
            tc.tile_pool(name="ps_tp", bufs=2, space=bass.MemorySpace.PSUM))
        ps_c2q = ctx.enter_context(
            tc.tile_pool(name="ps_c2q", bufs=2, space=bass.MemorySpace.PSUM))
        ps_acc = ctx.enter_context(
            tc.tile_pool(name="ps_acc", bufs=2, space=bass.MemorySpace.PSUM))

        # ---- once-per-core constants ----
        identity = singles.tile([P, P], f32)
        masks.make_identity(nc, identity[:])
        ones_row = singles.tile([1, P], f32)
        nc.vector.memset(ones_row, 1.0)
        ones_col = singles.tile([P, 1], f32)
        nc.vector.memset(ones_col, 1.0)

        wcols = {}
        for wname, wsrc in (("wh", wh_in), ("wu", wu_in), ("whu", whu_in)):
            t_ = singles.tile([100, 2], f32, tag=wname, name=wname)
            nc.sync.dma_start(out=t_[:, :], in_=wsrc.rearrange("(k p) -> p k", p=100))
            for kc in range(2):
                wcols[(wname, kc)] = t_[:, kc:kc + 1]

        loaded = {}

        def load_body(b):
            # u first (small; unblocks u-prep), then h
            u_sb = upool.tile([J, D], f32, tag="u", name="u_sb")
            nc.sync.dma_start(out=u_sb[:, 0:D], in_=u_in[b, :, :])
            htile = hpool.tile([P, 7 * D], f32, tag="h", name="htile")
            nmain = 6
            nc.sync.dma_start(
                out=htile[:, 0:nmain * D].rearrange("p (n d) -> p n d", d=D),
                in_=h_in[b, 0:nmain * P, :].rearrange("(n p) d -> p n d", p=P),
            )
            nc.sync.dma_start(
                out=htile[0:T - nmain * P, nmain * D:7 * D],
                in_=h_in[b, nmain * P:T, :],
            )
            loaded[b] = (u_sb, htile)

        def batch_body(b, prefetch=None):
            u_sb, htile = loaded.pop(b)

            # ---- u transposes, su row, s-matmul rhs ----
            uT = []
            for kc, (d0, kn) in enumerate(KCHUNKS):
                tp = ps_tp.tile([100, P], f32, tag="tp")
                nc.tensor.transpose(tp[:kn, :J], u_sb[:J, d0:d0 + kn], identity[:J, :J])
                uT_sb = upool.tile([100, J], f32, tag=f"uT{kc}")
                nc.scalar.copy(out=uT_sb[:kn, :], in_=tp[:kn, :J])
                uT.append(uT_sb)

            su_ps = ps_acc.tile([1, J], f32, tag="acc", name="su_ps")
            for kc, (d0, kn) in enumerate(KCHUNKS):
                nc.tensor.matmul(su_ps[:1, :], lhsT=wcols[("wu", kc)],
                                 rhs=uT[kc][:KCHUNKS[kc][1], :],
                                 start=(kc == 0), stop=(kc == 1))
            su_sb = bsmall.tile([1, J + 1], f32, tag="su")
            nc.vector.memset(su_sb, 0.0)
            nc.scalar.copy(out=su_sb[:1, 0:J], in_=su_ps[:1, :])

            rhs_ext = []
            for kc, (d0, kn) in enumerate(KCHUNKS):
                re_ = upool.tile([100, J + 1], f32, tag=f"rhs{kc}", name=f"rhs{kc}")
                nc.vector.tensor_scalar_mul(
                    out=re_[:kn, 0:J], in0=uT[kc][:kn, :], scalar1=wcols[("whu", kc)])
                nc.gpsimd.tensor_copy(out=re_[:kn, J:J + 1], in_=wcols[("wh", kc)])
                rhs_ext.append(re_)

            # ---- h transpose: hT [101, 2*800]; row 100 = ones (su path) ----
            hT = hTpool.tile([100, 2 * T], f32, tag="hT")
            for c, (t0, rows) in enumerate(TCHUNKS):
                tp = ps_tp.tile([100, 2 * P], f32, tag="tp")
                for kc, (d0, kn) in enumerate(KCHUNKS):
                    nc.tensor.matmul(
                        tp[:kn, kc * P:kc * P + rows],
                        lhsT=htile[:rows, c * D + d0:c * D + d0 + kn],
                        rhs=identity[:rows, :rows], is_transpose=True,
                        skip_group_check=True)
                nc.scalar.copy(
                    out=hT[:100, :].rearrange("p (k t) -> p k t", k=2)[:, :, t0:t0 + rows],
                    in_=tp[:100, :].rearrange("p (k c) -> p k c", k=2)[:, :, :rows])

            e_all = bsmall.tile([P, 7], f32, tag="e_all")
            nc.gpsimd.memset(e_all, 0.0)
            y_ps = ps_acc.tile([1, D], f32, tag="acc", name="y_ps")

            # ---- main chunk loop: chunks processed in pairs so the
            # softmax reductions/exp/copies run as one [128, 2, 50] op each ----
            assert not F32R
            for pair in ((0, 1), (2, 3), (4, 5), (6,)):
                c0 = pair[0]
                k = len(pair)
                t00 = c0 * P
                rr = TCHUNKS[pair[-1]][1]   # 128 for full pairs, 32 for (6,)
                s2 = ps_s.tile([P, 2 * (J + 1)], f32, tag="s", name="s2")
                for i, c in enumerate(pair):
                    t0, rows = TCHUNKS[c]
                    so = i * (J + 1)
                    nc.tensor.matmul(s2[:rows, so:so + J + 1],
                                     lhsT=hT[0:100, t0:t0 + rows],
                                     rhs=rhs_ext[0][:100, :], start=True, stop=False)
                    nc.tensor.matmul(s2[:rows, so:so + J + 1],
                                     lhsT=hT[0:100, T + t0:T + t0 + rows],
                                     rhs=rhs_ext[1][:100, :], start=False, stop=False)
                    nc.tensor.matmul(s2[:rows, so:so + J + 1],
                                     lhsT=ones_row[:1, :rows],
                                     rhs=su_sb[:1, :], start=False, stop=True)

                s2v = s2[:rr, :].rearrange("p (k j) -> p k j", j=J + 1)[:, 0:k, :]
                m2 = csmall.tile([P, 2], f32, tag="m")
                nc.vector.reduce_max(out=m2[:rr, 0:k], in_=s2v[:, :, 0:J], axis=AX.X)
                msh2 = csmall.tile([P, 2], f32, tag="msh")
                nc.vector.tensor_add(
                    out=msh2[:rr, 0:k].rearrange("p (k one) -> p k one", one=1),
                    in0=m2[:rr, 0:k].rearrange("p (k one) -> p k one", one=1),
                    in1=s2v[:, :, J:J + 1])
                nc.scalar.activation(out=e_all[:rr, c0:c0 + k], in_=msh2[:rr, 0:k],
                                     func=AF.Exp)
                p2 = csmall.tile([P, 2 * J], f32, tag="p", name="p2")
                nc.scalar.activation(
                    out=p2[:rr, 0:k * J].rearrange("p (k j) -> p k j", j=J),
                    in_=s2v[:, :, 0:J], func=AF.Exp)
                rs2 = csmall.tile([P, 2], f32, tag="rsum")
                nc.vector.reduce_sum(
                    out=rs2[:rr, 0:k],
                    in_=p2[:rr, 0:k * J].rearrange("p (k j) -> p k j", j=J),
                    axis=AX.X)
                rcp2 = csmall.tile([P, 2], f32, tag="rcp")
                nc.vector.reciprocal(out=rcp2[:rr, 0:k], in_=rs2[:rr, 0:k])

                tp2 = ps_tp.tile([100, 2 * P], f32, tag="tp")
                for i, c in enumerate(pair):
                    nc.tensor.matmul(tp2[:J, i * P:i * P + rr],
                                     lhsT=p2[:rr, i * J:(i + 1) * J],
                                     rhs=identity[:rr, :rr], is_transpose=True,
                                     skip_group_check=True)
                pT2 = csmall.tile([J, 2 * P], f32, tag="pT", name="pT2")
                nc.vector.tensor_copy(out=pT2[:J, 0:(k - 1) * P + rr],
                                      in_=tp2[:J, 0:(k - 1) * P + rr])

                gt = gpool.tile([P, 6 * D], f32, tag="g", name="gt")
                gv = gt[:rr, 0:k * 3 * D].rearrange("p (k x) -> p k x", x=3 * D)
                hv = htile[:rr, c0 * D:(c0 + k) * D].rearrange("p (k d) -> p k d", d=D)
                nc.gpsimd.tensor_copy(out=gv[:, :, 0:D], in_=hv)
                for i, c in enumerate(pair):
                    cps = ps_c2q.tile([P, D], f32, tag="c2q")
                    nc.tensor.matmul(cps[:rr, :], lhsT=pT2[:J, i * P:i * P + rr],
                                     rhs=u_sb[:J, 0:D], start=True, stop=True)
                    nc.tensor.matmul(y_ps[:1, :], lhsT=e_all[:rr, c:c + 1],
                                     rhs=htile[:rr, c * D:(c + 1) * D],
                                     start=(c == 0), stop=(c == 6))
                    nc.vector.tensor_scalar_mul(
                        out=gt[:rr, i * 3 * D + D:i * 3 * D + 2 * D],
                        in0=cps[:rr, :], scalar1=rcp2[:rr, i:i + 1])
                nc.gpsimd.tensor_mul(out=gv[:, :, 2 * D:3 * D], in0=hv,
                                     in1=gv[:, :, D:2 * D])
                if k == 2:
                    nc.sync.dma_start(
                        out=g_out[b, t00:t00 + 2 * P, 0:3 * D].rearrange(
                            "(k p) x -> p k x", p=P),
                        in_=gt[:, :].rearrange("p (k x) -> p k x", k=2))
                else:
                    nc.sync.dma_start(out=g_out[b, t00:t00 + rr, 0:3 * D],
                                      in_=gt[:rr, 0:3 * D])

            # issue next batch's input DMAs before the tail: the SP ring is
            # FIFO and the hq DMAs below wait on the q2c chain
            if prefetch is not None:
                prefetch()

            # ---- batch tail: q2c ----
            S_ps = ps_acc.tile([1, 7], f32, tag="acc")
            nc.tensor.matmul(S_ps[:1, :], lhsT=ones_col[:P, :1], rhs=e_all[:, :],
                             start=True, stop=True)
            Ssum = bsmall.tile([1, 1], f32, tag="Ssum")
            nc.vector.reduce_sum(out=Ssum[:1, :], in_=S_ps[:1, :], axis=AX.X)
            Sinv = bsmall.tile([1, 1], f32, tag="Sinv")
            nc.vector.reciprocal(out=Sinv[:1, :], in_=Ssum[:1, :])
            q2c_sb = bsmall.tile([1, D], f32, tag="q2c")
            nc.vector.tensor_scalar_mul(out=q2c_sb[:1, :], in0=y_ps[:1, 0:D],
                                        scalar1=Sinv[:1, :])
            q2cb_ps = ps_acc.tile([P, D], f32, tag="acc")
            nc.tensor.matmul(q2cb_ps[:, :], lhsT=ones_row[:1, :], rhs=q2c_sb[:1, :],
                             start=True, stop=True)
            q2cb_sb = bsmall.tile([P, D], f32, tag="q2cb")
            nc.scalar.copy(out=q2cb_sb[:, :], in_=q2cb_ps[:, :])

            hq_all = hqpool.tile([P, 7 * D], f32, tag="hq")
            q2cb_b3 = bass.AP(tensor=q2cb_sb.tensor, offset=q2cb_sb.offset,
                              ap=[q2cb_sb.ap[0], [0, 3], q2cb_sb.ap[1]])
            nc.vector.tensor_mul(
                out=hq_all[:, 0:3 * D].rearrange("p (n d) -> p n d", d=D),
                in0=htile[:, 0:3 * D].rearrange("p (n d) -> p n d", d=D),
                in1=q2cb_b3)
            nc.gpsimd.tensor_mul(
                out=hq_all[:, 3 * D:6 * D].rearrange("p (n d) -> p n d", d=D),
                in0=htile[:, 3 * D:6 * D].rearrange("p (n d) -> p n d", d=D),
                in1=q2cb_b3)
            nc.vector.tensor_mul(out=hq_all[0:T - 6 * P, 6 * D:7 * D],
                                 in0=htile[0:T - 6 * P, 6 * D:7 * D],
                                 in1=q2cb_sb[0:T - 6 * P, :])
            nc.sync.dma_start(
                out=g_out[b, 0:6 * P, 3 * D:4 * D].rearrange("(n p) d -> p n d", p=P),
                in_=hq_all[:, 0:6 * D].rearrange("p (n d) -> p n d", d=D))
            nc.sync.dma_start(out=g_out[b, 6 * P:T, 3 * D:4 * D],
                              in_=hq_all[0:T - 6 * P, 6 * D:7 * D])

        def run_all():
            load_body(0)
            for b in range(BC):
                pf = (lambda nb=b + 1: load_body(nb)) if b + 1 < BC else None
                batch_body(b, prefetch=pf)

        if reps == 1:
            run_all()
        else:
            with tc.For_i(0, reps, 1):
                run_all()

    return nc


def kernel(h, u, w_h, b_h, w_u, b_u, w_hu, b_hu):
    from concourse.bass_utils import run_bass_kernel_spmd

    if "nc" not in _cache:
        nc = _build()
        _split_multi_waits(nc)
        _cache["nc"] = nc
    nc = _cache["nc"]

    h = np.ascontiguousarray(h, dtype=np.float32)
    u = np.ascontiguousarray(u, dtype=np.float32)
    w_h = np.ascontiguousarray(w_h, dtype=np.float32)
    w_u = np.ascontiguousarray(w_u, dtype=np.float32)
    w_hu = np.ascontiguousarray(w_hu, dtype=np.float32)

    core_ids = list(range(NCORES))
    in_maps = []
    for i in core_ids:
        in_maps.append({
            "h": h[i * BC:(i + 1) * BC],
            "u": u[i * BC:(i + 1) * BC],
            "w_h": w_h,
            "w_u": w_u,
            "w_hu": w_hu,
        })
    res = run_bass_kernel_spmd(nc, in_maps, core_ids)
    _cache["last_results"] = res
    return np.concatenate([res.results[i]["g"] for i in core_ids], axis=0)


# revision 32
# speedup vs baseline: 3.2262x; 3.2262x over previous
"""BiDAF attention-flow layer on 8 Trainium2 NeuronCores.

Data-parallel over batch: each core processes B/8 = 8 batches.

Math (per batch b):
  s[t,j] = h[t]·w_h + u[j]·w_u + (h[t]*w_hu)·u[j] + const
  a      = softmax_j(s)            -> only needs  sj = shu + su  (row consts cancel)
  c2q    = a @ u
  bt     = softmax_t(max_j s)      -> needs  m + sh  where m = max_j(sj)
  q2c    = bt @ h
  g      = [h | c2q | h*c2q | h*q2c]

The rank-1 bias terms b_h/b_u/b_hu shift every s[t,j] equally and cancel in
both softmaxes, so they are accepted but unused.

Layout per batch (core-local):
  htile [128, 7*200]   h rows chunked by 128 (chunk c at cols c*200..)
  hT    [100, 2*800]   h transposed (PE transpose pairs, one fused copy)
  s_ps  [128, 51]      cols 0:50 = shu+su (su via K=1 ones-matmul), col 50 = sh
  softmax on free dim; p transposed back (PE) for the c2q matmul;
  y[1,200] = sum_t e_t h_t accumulated in PSUM over chunks; q2c = y/sum(e);
  g cols 0:600 stream out as chunk-pair DMAs, cols 600:800 (needs q2c) as
  two per-batch DMAs. Engines balanced: ACT exp/copies, DVE softmax+scales,
  GPSIMD h-copy + h*c2q, PE matmuls/transposes; all DMAs on the SP HWDGE ring.
"""
import sys

if '/opt/trn_rl_repo' not in sys.path:
    sys.path.insert(0, '/opt/trn_rl_repo')

import numpy as np

B, T, J, D = 64, 800, 50, 200
NCORES = 8
BC = B // NCORES            # batches per core
P = 128
TCHUNKS = [(c * P, min(P, T - c * P)) for c in range((T + P - 1) // P)]
KCHUNKS = [(0, 100), (100, 100)]
NPAD = 256

_cache = {}
F32R = False  # f32r c2q is ~8us faster but 50x less accurate; keep exact


def _split_multi_waits(nc, max_waits=1):
    """This walrus build accepts at most one sync-wait per instruction.
    For any instruction carrying more, move the extra waits onto pure-wait
    EventSemaphore carriers inserted just before it on the same engine —
    the sequencer dispatches in order, so the blocking behavior is
    identical."""
    from concourse import mybir
    import bass_rust
    n = 0
    for f in nc.m.functions:
        for blk in f.blocks:
            insts = blk.instructions
            i = 0
            while i < len(insts):
                inst = insts[i]
                si = inst.sync_info
                if si is not None and len(si.on_wait) > max_waits:
                    waits = list(si.on_wait)
                    keep = waits[-max_waits:]
                    new = []
                    for w in waits[:-max_waits]:
                        d = mybir.InstEventSemaphore(
                            name=f"{inst.name}-sw{n}", ins=[], outs=[])
                        n += 1
                        d.engine = inst.engine
                        d.sync_info = bass_rust.SyncInfo(on_wait=[w], on_update=[])
                        new.append(d)
                    inst.sync_info = bass_rust.SyncInfo(
                        on_wait=keep, on_update=list(si.on_update))
                    for j, d in enumerate(new):
                        insts.insert(i + j, d)
                    i += len(new)
                i += 1
    return n


def _build(reps=1):
    import concourse.bass as bass
    import concourse.tile as tile
    from concourse import mybir, masks
    from contextlib import ExitStack

    f32 = mybir.dt.float32
    f32r = mybir.dt.float32r
    AF = mybir.ActivationFunctionType
    AX = mybir.AxisListType

    nc = bass.Bass()
    h_in = nc.declare_dram_parameter("h", [BC, T, D], f32, isOutput=False)
    u_in = nc.declare_dram_parameter("u", [BC, J, D], f32, isOutput=False)
    wh_in = nc.declare_dram_parameter("w_h", [D], f32, isOutput=False)
    wu_in = nc.declare_dram_parameter("w_u", [D], f32, isOutput=False)
    whu_in = nc.declare_dram_parameter("w_hu", [D], f32, isOutput=False)
    g_out = nc.declare_dram_parameter("g", [BC, T, 4 * D], f32, isOutput=True)

    with tile.TileContext(nc) as tc, ExitStack() as ctx:
        singles = ctx.enter_context(tc.tile_pool(name="singles", bufs=1))
        hpool = ctx.enter_context(tc.tile_pool(name="hpool", bufs=4))
        hTpool = ctx.enter_context(tc.tile_pool(name="hTpool", bufs=3))
        upool = ctx.enter_context(tc.tile_pool(name="upool", bufs=3))
        bsmall = ctx.enter_context(tc.tile_pool(name="bsmall", bufs=4))
        csmall = ctx.enter_context(tc.tile_pool(name="csmall", bufs=6))
        gpool = ctx.enter_context(tc.tile_pool(name="gpool", bufs=8))
        hqpool = ctx.enter_context(tc.tile_pool(name="hqpool", bufs=3))
        ps_s = ctx.enter_context(
            tc.tile_pool(name="ps_s", bufs=2, space=bass.MemorySpace.PSUM))
        ps_tp = ctx.enter_context(
            tc.tile_pool(name="ps_tp", bufs=2, space=bass.MemorySpace.PSUM))
        ps_c2q = ctx.enter_context(
            tc.tile_pool(name="ps_c2q", bufs=2, space=bass.MemorySpace.PSUM))
        ps_acc = ctx.enter_context(
            tc.tile_pool(name="ps_acc", bufs=2, space=bass.MemorySpace.PSUM))

        # ---- once-per-core constants ----
        identity = singles.tile([P, P], f32)
        masks.make_identity(nc, identity[:])
        ones_row = singles.tile([1, P], f32)
        nc.vector.memset(ones_row, 1.0)
        ones_col = singles.tile([P, 1], f32)
        nc.vector.memset(ones_col, 1.0)

        wcols = {}
        for wname, wsrc in (("wh", wh_in), ("wu", wu_in), ("whu", whu_in)):
            t_ = singles.tile([100, 2], f32, tag=wname, name=wname)
            nc.sync.dma_start(out=t_[:, :], in_=wsrc.rearrange("(k p) -> p k", p=100))
            for kc in range(2):
                wcols[(wname, kc)] = t_[:, kc:kc + 1]

        loaded = {}

        def load_body(b):
            # u first (small; unblocks u-prep), then h
            u_sb = upool.tile([J, D], f32, tag="u", name="u_sb")
            nc.sync.dma_start(out=u_sb[:, 0:D], in_=u_in[b, :, :])
            htile = hpool.tile([P, 7 * D], f32, tag="h", name="htile")
            nmain = 6
            nc.sync.dma_start(
                out=htile[:, 0:nmain * D].rearrange("p (n d) -> p n d", d=D),
                in_=h_in[b, 0:nmain * P, :].rearrange("(n p) d -> p n d", p=P),
            )
            nc.sync.dma_start(
                out=htile[0:T - nmain * P, nmain * D:7 * D],
                in_=h_in[b, nmain * P:T, :],
            )
            loaded[b] = (u_sb, htile)

        def batch_body(b, prefetch=None):
            u_sb, htile = loaded.pop(b)

            # ---- u transposes, su row, s-matmul rhs ----
            uT = []
            for kc, (d0, kn) in enumerate(KCHUNKS):
                tp = ps_tp.tile([100, P], f32, tag="tp")
                nc.tensor.transpose(tp[:kn, :J], u_sb[:J, d0:d0 + kn], identity[:J, :J])
                uT_sb = upool.tile([100, J], f32, tag=f"uT{kc}")
                nc.scalar.copy(out=uT_sb[:kn, :], in_=tp[:kn, :J])
                uT.append(uT_sb)

            su_ps = ps_acc.tile([1, J], f32, tag="acc", name="su_ps")
            for kc, (d0, kn) in enumerate(KCHUNKS):
                nc.tensor.matmul(su_ps[:1, :], lhsT=wcols[("wu", kc)],
                                 rhs=uT[kc][:KCHUNKS[kc][1], :],
                                 start=(kc == 0), stop=(kc == 1))
            su_sb = bsmall.tile([1, J + 1], f32, tag="su")
            nc.vector.memset(su_sb, 0.0)
            nc.scalar.copy(out=su_sb[:1, 0:J], in_=su_ps[:1, :])

            rhs_ext = []
            for kc, (d0, kn) in enumerate(KCHUNKS):
                re_ = upool.tile([100, J + 1], f32, tag=f"rhs{kc}", name=f"rhs{kc}")
                nc.vector.tensor_scalar_mul(
                    out=re_[:kn, 0:J], in0=uT[kc][:kn, :], scalar1=wcols[("whu", kc)])
                nc.gpsimd.tensor_copy(out=re_[:kn, J:J + 1], in_=wcols[("wh", kc)])
                rhs_ext.append(re_)

            # ---- h transpose: hT [101, 2*800]; row 100 = ones (su path) ----
            hT = hTpool.tile([100, 2 * T], f32, tag="hT")
            for c, (t0, rows) in enumerate(TCHUNKS):
                tp = ps_tp.tile([100, 2 * P], f32, tag="tp")
                for kc, (d0, kn) in enumerate(KCHUNKS):
                    nc.tensor.matmul(
                        tp[:kn, kc * P:kc * P + rows],
                        lhsT=htile[:rows, c * D + d0:c * D + d0 + kn],
                        rhs=identity[:rows, :rows], is_transpose=True,
                        skip_group_check=True)
                nc.scalar.copy(
                    out=hT[:100, :].rearrange("p (k t) -> p k t", k=2)[:, :, t0:t0 + rows],
                    in_=tp[:100, :].rearrange("p (k c) -> p k c", k=2)[:, :, :rows])

            e_all = bsmall.tile([P, 7], f32, tag="e_all")
            nc.gpsimd.memset(e_all, 0.0)
            y_ps = ps_acc.tile([1, D], f32, tag="acc", name="y_ps")

            # ---- main chunk loop: chunks processed in pairs so the
            # softmax reductions/exp/copies run as one [128, 2, 50] op each ----
            assert not F32R
            for pair in ((0, 1), (2, 3), (4, 5), (6,)):
                c0 = pair[0]
                k = len(pair)
                t00 = c0 * P
                rr = TCHUNKS[pair[-1]][1]   # 128 for full pairs, 32 for (6,)
                s2 = ps_s.tile([P, 2 * (J + 1)], f32, tag="s", name="s2")
                for i, c in enumerate(pair):
                    t0, rows = TCHUNKS[c]
                    so = i * (J + 1)
                    nc.tensor.matmul(s2[:rows, so:so + J + 1],
                                     lhsT=hT[0:100, t0:t0 + rows],
                                     rhs=rhs_ext[0][:100, :], start=True, stop=False)
                    nc.tensor.matmul(s2[:rows, so:so + J + 1],
                                     lhsT=hT[0:100, T + t0:T + t0 + rows],
                                     rhs=rhs_ext[1][:100, :], start=False, stop=False)
                    nc.tensor.matmul(s2[:rows, so:so + J + 1],
                                     lhsT=ones_row[:1, :rows],
                                     rhs=su_sb[:1, :], start=False, stop=True)

                s2v = s2[:rr, :].rearrange("p (k j) -> p k j", j=J + 1)[:, 0:k, :]
                m2 = csmall.tile([P, 2], f32, tag="m")
                nc.vector.reduce_max(out=m2[:rr, 0:k], in_=s2v[:, :, 0:J], axis=AX.X)
                msh2 = csmall.tile([P, 2], f32, tag="msh")
                nc.vector.tensor_add(
                    out=msh2[:rr, 0:k].rearrange("p (k one) -> p k one", one=1),
                    in0=m2[:rr, 0:k].rearrange("p (k one) -> p k one", one=1),
                    in1=s2v[:, :, J:J + 1])
                nc.scalar.activation(out=e_all[:rr, c0:c0 + k], in_=msh2[:rr, 0:k],
                                     func=AF.Exp)
                p2 = csmall.tile([P, 2 * J], f32, tag="p", name="p2")
                nc.scalar.activation(
                    out=p2[:rr, 0:k * J].rearrange("p (k j) -> p k j", j=J),
                    in_=s2v[:, :, 0:J], func=AF.Exp)
                rs2 = csmall.tile([P, 2], f32, tag="rsum")
                nc.vector.reduce_sum(
                    out=rs2[:rr, 0:k],
                    in_=p2[:rr, 0:k * J].rearrange("p (k j) -> p k j", j=J),
                    axis=AX.X)
                rcp2 = csmall.tile([P, 2], f32, tag="rcp")
                nc.vector.reciprocal(out=rcp2[:rr, 0:k], in_=rs2[:rr, 0:k])

                tp2 = ps_tp.tile([100, 2 * P], f32, tag="tp")
                for i, c in enumerate(pair):
                    nc.tensor.matmul(tp2[:J, i * P:i * P + rr],
                                     lhsT=p2[:rr, i * J:(i + 1) * J],
                                     rhs=identity[:rr, :rr], is_transpose=True,
                                     skip_group_check=True)
                pT2 = csmall.tile([J, 2 * P], f32, tag="pT", name="pT2")
                nc.vector.tensor_copy(out=pT2[:J, 0:(k - 1) * P + rr],
                                      in_=tp2[:J, 0:(k - 1) * P + rr])

                gt = gpool.tile([P, 6 * D], f32, tag="g", name="gt")
                gv = gt[:rr, 0:k * 3 * D].rearrange("p (k x) -> p k x", x=3 * D)
                hv = htile[:rr, c0 * D:(c0 + k) * D].rearrange("p (k d) -> p k d", d=D)
                nc.gpsimd.tensor_copy(out=gv[:, :, 0:D], in_=hv)
                cps = ps_c2q.tile([P, 2 * D], f32, tag="c2q", name="cps")
                for i, c in enumerate(pair):
                    nc.tensor.matmul(cps[:rr, i * D:(i + 1) * D],
                                     lhsT=pT2[:J, i * P:i * P + rr],
                                     rhs=u_sb[:J, 0:D], start=True, stop=True,
                                     skip_group_check=True)
                    nc.tensor.matmul(y_ps[:1, :], lhsT=e_all[:rr, c:c + 1],
                                     rhs=htile[:rr, c * D:(c + 1) * D],
                                     start=(c == 0), stop=(c == 6))
                rcp_s = rcp2[:rr, 0:k]
                rcp_b = bass.AP(tensor=rcp_s.tensor, offset=rcp_s.offset,
                                ap=[rcp_s.ap[0][:], [1, k], [0, D]])
                nc.vector.tensor_mul(
                    out=gv[:, :, D:2 * D],
                    in0=cps[:rr, 0:k * D].rearrange("p (k d) -> p k d", d=D),
                    in1=rcp_b)
                nc.gpsimd.tensor_mul(out=gv[:, :, 2 * D:3 * D], in0=hv,
                                     in1=gv[:, :, D:2 * D])
                if k == 2:
                    nc.sync.dma_start(
                        out=g_out[b, t00:t00 + 2 * P, 0:3 * D].rearrange(
                            "(k p) x -> p k x", p=P),
                        in_=gt[:, :].rearrange("p (k x) -> p k x", k=2))
                else:
                    nc.sync.dma_start(out=g_out[b, t00:t00 + rr, 0:3 * D],
                                      in_=gt[:rr, 0:3 * D])

            # issue next batch's input DMAs before the tail: the SP ring is
            # FIFO and the hq DMAs below wait on the q2c chain
            if prefetch is not None:
                prefetch()

            # ---- batch tail: q2c ----
            S_ps = ps_acc.tile([1, 7], f32, tag="acc")
            nc.tensor.matmul(S_ps[:1, :], lhsT=ones_col[:P, :1], rhs=e_all[:, :],
                             start=True, stop=True)
            Ssum = bsmall.tile([1, 1], f32, tag="Ssum")
            nc.vector.reduce_sum(out=Ssum[:1, :], in_=S_ps[:1, :], axis=AX.X)
            Sinv = bsmall.tile([1, 1], f32, tag="Sinv")
            nc.vector.reciprocal(out=Sinv[:1, :], in_=Ssum[:1, :])
            q2c_sb = bsmall.tile([1, D], f32, tag="q2c")
            nc.vector.tensor_scalar_mul(out=q2c_sb[:1, :], in0=y_ps[:1, 0:D],
                                        scalar1=Sinv[:1, :])
            q2cb_ps = ps_acc.tile([P, D], f32, tag="acc")
            nc.tensor.matmul(q2cb_ps[:, :], lhsT=ones_row[:1, :], rhs=q2c_sb[:1, :],
                             start=True, stop=True)
            q2cb_sb = bsmall.tile([P, D], f32, tag="q2cb")
            nc.scalar.copy(out=q2cb_sb[:, :], in_=q2cb_ps[:, :])

            hq_all = hqpool.tile([P, 7 * D], f32, tag="hq")
            q2cb_b3 = bass.AP(tensor=q2cb_sb.tensor, offset=q2cb_sb.offset,
                              ap=[q2cb_sb.ap[0], [0, 3], q2cb_sb.ap[1]])
            nc.vector.tensor_mul(
                out=hq_all[:, 0:3 * D].rearrange("p (n d) -> p n d", d=D),
                in0=htile[:, 0:3 * D].rearrange("p (n d) -> p n d", d=D),
                in1=q2cb_b3)
            nc.gpsimd.tensor_mul(
                out=hq_all[:, 3 * D:6 * D].rearrange("p (n d) -> p n d", d=D),
                in0=htile[:, 3 * D:6 * D].rearrange("p (n d) -> p n d", d=D),
                in1=q2cb_b3)
            nc.vector.tensor_mul(out=hq_all[0:T - 6 * P, 6 * D:7 * D],
                                 in0=htile[0:T - 6 * P, 6 * D:7 * D],
                                 in1=q2cb_sb[0:T - 6 * P, :])
            nc.sync.dma_start(
                out=g_out[b, 0:6 * P, 3 * D:4 * D].rearrange("(n p) d -> p n d", p=P),
                in_=hq_all[:, 0:6 * D].rearrange("p (n d) -> p n d", d=D))
            nc.sync.dma_start(out=g_out[b, 6 * P:T, 3 * D:4 * D],
                              in_=hq_all[0:T - 6 * P, 6 * D:7 * D])

        def run_all():
            load_body(0)
            for b in range(BC):
                pf = (lambda nb=b + 1: load_body(nb)) if b + 1 < BC else None
                batch_body(b, prefetch=pf)

        if reps == 1:
            run_all()
        else:
            with tc.For_i(0, reps, 1):
                run_all()

    return nc


def kernel(h, u, w_h, b_h, w_u, b_u, w_hu, b_hu):
    from concourse.bass_utils import run_bass_kernel_spmd

    if "nc" not in _cache:
        nc = _build()
        _split_multi_waits(nc)
        _cache["nc"] = nc
    nc = _cache["nc"]

    h = np.ascontiguousarray(h, dtype=np.float32)
    u = np.ascontiguousarray(u, dtype=np.float32)
    w_h = np.ascontiguousarray(w_h, dtype=np.float32)
    w_u = np.ascontiguousarray(w_u, dtype=np.float32)
    w_hu = np.ascontiguousarray(w_hu, dtype=np.float32)

    core_ids = list(range(NCORES))
    in_maps = []
    for i in core_ids:
        in_maps.append({
            "h": h[i * BC:(i + 1) * BC],
            "u": u[i * BC:(i + 1) * BC],
            "w_h": w_h,
            "w_u": w_u,
            "w_hu": w_hu,
        })
    res = run_bass_kernel_spmd(nc, in_maps, core_ids)
    _cache["last_results"] = res
    return np.concatenate([res.results[i]["g"] for i in core_ids], axis=0)


# revision 34
# speedup vs baseline: 7.5833x; 2.3505x over previous
"""BiDAF attention-flow layer on 8 Trainium2 NeuronCores.

Data-parallel over batch: each core processes B/8 = 8 batches.

Math (per batch b):
  s[t,j] = h[t]·w_h + u[j]·w_u + (h[t]*w_hu)·u[j] + const
  a      = softmax_j(s)            -> only needs  sj = shu + su  (row consts cancel)
  c2q    = a @ u
  bt     = softmax_t(max_j s)      -> needs  m + sh  where m = max_j(sj)
  q2c    = bt @ h
  g      = [h | c2q | h*c2q | h*q2c]

The rank-1 bias terms b_h/b_u/b_hu shift every s[t,j] equally and cancel in
both softmaxes, so they are accepted but unused.

Layout per batch (core-local):
  htile [128, 7*200]   h rows chunked by 128 (chunk c at cols c*200..)
  hT    [100, 2*800]   h transposed (PE transpose pairs, one fused copy)
  s_ps  [128, 51]      cols 0:50 = shu+su (su via K=1 ones-matmul), col 50 = sh
  softmax on free dim; p transposed back (PE) for the c2q matmul;
  y[1,200] = sum_t e_t h_t accumulated in PSUM over chunks; q2c = y/sum(e);
  g cols 0:600 stream out as chunk-pair DMAs, cols 600:800 (needs q2c) as
  two per-batch DMAs. Engines balanced: ACT exp/copies, DVE softmax+scales,
  GPSIMD h-copy + h*c2q, PE matmuls/transposes; all DMAs on the SP HWDGE ring.
"""
import sys

if '/opt/trn_rl_repo' not in sys.path:
    sys.path.insert(0, '/opt/trn_rl_repo')

import numpy as np

B, T, J, D = 64, 800, 50, 200
NCORES = 8
BC = B // NCORES            # batches per core
P = 128
TCHUNKS = [(c * P, min(P, T - c * P)) for c in range((T + P - 1) // P)]
KCHUNKS = [(0, 100), (100, 100)]
NPAD = 256
DS = 201  # htile chunk stride: 200 h cols + a ones column

_cache = {}
F32R = False  # f32r c2q is ~8us faster but 50x less accurate; keep exact


def _split_multi_waits(nc, max_waits=1):
    """This walrus build accepts at most one sync-wait per instruction.
    For any instruction carrying more, move the extra waits onto pure-wait
    EventSemaphore carriers inserted just before it on the same engine —
    the sequencer dispatches in order, so the blocking behavior is
    identical."""
    from concourse import mybir
    import bass_rust
    n = 0
    for f in nc.m.functions:
        for blk in f.blocks:
            insts = blk.instructions
            i = 0
            while i < len(insts):
                inst = insts[i]
                si = inst.sync_info
                if si is not None and len(si.on_wait) > max_waits:
                    waits = list(si.on_wait)
                    keep = waits[-max_waits:]
                    new = []
                    for w in waits[:-max_waits]:
                        d = mybir.InstEventSemaphore(
                            name=f"{inst.name}-sw{n}", ins=[], outs=[])
                        n += 1
                        d.engine = inst.engine
                        d.sync_info = bass_rust.SyncInfo(on_wait=[w], on_update=[])
                        new.append(d)
                    inst.sync_info = bass_rust.SyncInfo(
                        on_wait=keep, on_update=list(si.on_update))
                    for j, d in enumerate(new):
                        insts.insert(i + j, d)
                    i += len(new)
                i += 1
    return n


def _build(reps=1):
    import concourse.bass as bass
    import concourse.tile as tile
    from concourse import mybir, masks
    from contextlib import ExitStack

    f32 = mybir.dt.float32
    f32r = mybir.dt.float32r
    AF = mybir.ActivationFunctionType
    AX = mybir.AxisListType

    nc = bass.Bass()
    h_in = nc.declare_dram_parameter("h", [BC, T, D], f32, isOutput=False)
    u_in = nc.declare_dram_parameter("u", [BC, J, D], f32, isOutput=False)
    wh_in = nc.declare_dram_parameter("w_h", [D], f32, isOutput=False)
    wu_in = nc.declare_dram_parameter("w_u", [D], f32, isOutput=False)
    whu_in = nc.declare_dram_parameter("w_hu", [D], f32, isOutput=False)
    g_out = nc.declare_dram_parameter("g", [BC, T, 4 * D], f32, isOutput=True)

    with tile.TileContext(nc) as tc, ExitStack() as ctx:
        singles = ctx.enter_context(tc.tile_pool(name="singles", bufs=1))
        hpool = ctx.enter_context(tc.tile_pool(name="hpool", bufs=4))
        hTpool = ctx.enter_context(tc.tile_pool(name="hTpool", bufs=3))
        upool = ctx.enter_context(tc.tile_pool(name="upool", bufs=3))
        bsmall = ctx.enter_context(tc.tile_pool(name="bsmall", bufs=4))
        csmall = ctx.enter_context(tc.tile_pool(name="csmall", bufs=6))
        gpool = ctx.enter_context(tc.tile_pool(name="gpool", bufs=8))
        hqpool = ctx.enter_context(tc.tile_pool(name="hqpool", bufs=3))
        ps_s = ctx.enter_context(
            tc.tile_pool(name="ps_s", bufs=2, space=bass.MemorySpace.PSUM))
        ps_tp = ctx.enter_context(
            tc.tile_pool(name="ps_tp", bufs=2, space=bass.MemorySpace.PSUM))
        ps_c2q = ctx.enter_context(
            tc.tile_pool(name="ps_c2q", bufs=2, space=bass.MemorySpace.PSUM))
        ps_acc = ctx.enter_context(
            tc.tile_pool(name="ps_acc", bufs=2, space=bass.MemorySpace.PSUM))

        # ---- once-per-core constants ----
        identity = singles.tile([P, P], f32)
        masks.make_identity(nc, identity[:])
        ones_row = singles.tile([1, P], f32)
        nc.vector.memset(ones_row, 1.0)
        ones_col = singles.tile([P, 1], f32)
        nc.vector.memset(ones_col, 1.0)

        wh0 = singles.tile([96, 1], f32, tag="wh0", name="wh0")
        nc.sync.dma_start(out=wh0[:, :],
                          in_=wh_in[0:96].rearrange("(p one) -> p one", one=1))
        wh1e = singles.tile([105, 1], f32, tag="wh1e", name="wh1e")
        nc.vector.memset(wh1e, 0.0)
        nc.sync.dma_start(out=wh1e[0:104, :],
                          in_=wh_in[96:D].rearrange("(p one) -> p one", one=1))

        def bcast_load(wsrc, name):
            t_ = singles.tile([J, D], f32, tag=name, name=name)
            s_ap = wsrc[:]
            nc.sync.dma_start(out=t_[:, :], in_=bass.AP(
                tensor=s_ap.tensor, offset=s_ap.offset, ap=[[0, J], s_ap.ap[0][:]]))
            return t_

        whu_b = bcast_load(whu_in, "whu_b")
        wu_b = bcast_load(wu_in, "wu_b")

        loaded = {}

        def load_body(b):
            # u first (small; unblocks u-prep), then h
            u_sb = upool.tile([J, D], f32, tag="u", name="u_sb")
            nc.sync.dma_start(out=u_sb[:, 0:D], in_=u_in[b, :, :])
            htile = hpool.tile([P, 7 * DS], f32, tag="h", name="htile")
            nc.gpsimd.memset(
                htile[:, :].rearrange("p (n d) -> p n d", d=DS)[:, :, D:DS], 1.0)
            nmain = 6
            nc.sync.dma_start(
                out=htile[:, 0:nmain * DS].rearrange(
                    "p (n d) -> p n d", d=DS)[:, :, 0:D],
                in_=h_in[b, 0:nmain * P, :].rearrange("(n p) d -> p n d", p=P),
            )
            nc.sync.dma_start(
                out=htile[0:T - nmain * P, nmain * DS:nmain * DS + D],
                in_=h_in[b, nmain * P:T, :],
            )
            loaded[b] = (u_sb, htile)

        def batch_body(b, prefetch=None):
            load_body(b)
            u_sb, htile = loaded.pop(b)

            # ---- u-side prep: u_w = u*w_hu with su appended as col 200;
            # transposing u_w gives the s-matmul rhs (su lands in K-row 104,
            # paired with the ones column embedded in htile) ----
            u_w = upool.tile([J, DS], f32, tag="u_w", name="u_w")
            nc.vector.tensor_mul(out=u_w[:, 0:D], in0=u_sb[:, 0:D],
                                 in1=whu_b[:, :])
            su_t = upool.tile([J, D], f32, tag="su_t", name="su_t")
            nc.vector.tensor_mul(out=su_t[:, :], in0=u_sb[:, 0:D], in1=wu_b[:, :])
            su_col = bsmall.tile([J, 1], f32, tag="su")
            nc.vector.reduce_sum(out=su_col[:, :], in_=su_t[:, :], axis=AX.X)
            nc.gpsimd.tensor_copy(out=u_w[:, D:DS], in_=su_col[:, :])

            rhs_ext = []
            for kc, (d0, kn, whcol) in enumerate(((0, 96, wh0), (96, 105, wh1e))):
                tp = ps_tp.tile([105, P], f32, tag="tp")
                nc.tensor.transpose(tp[:kn, :J], u_w[:J, d0:d0 + kn],
                                    identity[:J, :J])
                re_ = upool.tile([105, J + 1], f32, tag=f"rhs{kc}", name=f"rhs{kc}")
                nc.scalar.copy(out=re_[:kn, 0:J], in_=tp[:kn, :J])
                nc.gpsimd.tensor_copy(out=re_[:kn, J:J + 1], in_=whcol[:kn, :])
                rhs_ext.append(re_)

            # ---- h transpose: hT [105, 2*800]; kc1 row 104 = ones (su) ----
            hT = hTpool.tile([105, 2 * T], f32, tag="hT")
            for c, (t0, rows) in enumerate(TCHUNKS):
                tp = ps_tp.tile([105, 2 * P], f32, tag="tp")
                for kc, d0 in enumerate((0, 96)):
                    nc.tensor.matmul(
                        tp[:105, kc * P:kc * P + rows],
                        lhsT=htile[:rows, c * DS + d0:c * DS + d0 + 105],
                        rhs=identity[:rows, :rows], is_transpose=True,
                        skip_group_check=True)
                nc.scalar.copy(
                    out=hT[:105, :].rearrange("p (k t) -> p k t", k=2)[:, :, t0:t0 + rows],
                    in_=tp[:105, :].rearrange("p (k c) -> p k c", k=2)[:, :, :rows])

            e_all = bsmall.tile([P, 7], f32, tag="e_all")
            nc.gpsimd.memset(e_all, 0.0)
            y_ps = ps_acc.tile([1, D], f32, tag="acc", name="y_ps")

            # ---- main chunk loop: chunks processed in pairs so the
            # softmax reductions/exp/copies run as one [128, 2, 50] op each ----
            assert not F32R
            for pair in ((0, 1), (2, 3), (4, 5), (6,)):
                c0 = pair[0]
                k = len(pair)
                t00 = c0 * P
                rr = TCHUNKS[pair[-1]][1]   # 128 for full pairs, 32 for (6,)
                s2 = ps_s.tile([P, 2 * (J + 1)], f32, tag="s", name="s2")
                for i, c in enumerate(pair):
                    t0, rows = TCHUNKS[c]
                    so = i * (J + 1)
                    nc.tensor.matmul(s2[:rows, so:so + J + 1],
                                     lhsT=hT[0:96, t0:t0 + rows],
                                     rhs=rhs_ext[0][:96, :], start=True, stop=False)
                    nc.tensor.matmul(s2[:rows, so:so + J + 1],
                                     lhsT=hT[0:105, T + t0:T + t0 + rows],
                                     rhs=rhs_ext[1][:105, :], start=False, stop=True)

                s2v = s2[:rr, :].rearrange("p (k j) -> p k j", j=J + 1)[:, 0:k, :]
                m2 = csmall.tile([P, 2], f32, tag="m")
                nc.vector.reduce_max(out=m2[:rr, 0:k], in_=s2v[:, :, 0:J], axis=AX.X)
                msh2 = csmall.tile([P, 2], f32, tag="msh")
                nc.vector.tensor_add(
                    out=msh2[:rr, 0:k].rearrange("p (k one) -> p k one", one=1),
                    in0=m2[:rr, 0:k].rearrange("p (k one) -> p k one", one=1),
                    in1=s2v[:, :, J:J + 1])
                nc.scalar.activation(out=e_all[:rr, c0:c0 + k], in_=msh2[:rr, 0:k],
                                     func=AF.Exp)
                p2 = csmall.tile([P, 2 * J], f32, tag="p", name="p2")
                nc.scalar.activation(
                    out=p2[:rr, 0:k * J].rearrange("p (k j) -> p k j", j=J),
                    in_=s2v[:, :, 0:J], func=AF.Exp)
                rs2 = csmall.tile([P, 2], f32, tag="rsum")
                nc.vector.reduce_sum(
                    out=rs2[:rr, 0:k],
                    in_=p2[:rr, 0:k * J].rearrange("p (k j) -> p k j", j=J),
                    axis=AX.X)
                rcp2 = csmall.tile([P, 2], f32, tag="rcp")
                nc.vector.reciprocal(out=rcp2[:rr, 0:k], in_=rs2[:rr, 0:k])

                tp2 = ps_tp.tile([100, 2 * P], f32, tag="tp")
                for i, c in enumerate(pair):
                    nc.tensor.matmul(tp2[:J, i * P:i * P + rr],
                                     lhsT=p2[:rr, i * J:(i + 1) * J],
                                     rhs=identity[:rr, :rr], is_transpose=True,
                                     skip_group_check=True)
                pT2 = csmall.tile([J, 2 * P], f32, tag="pT", name="pT2")
                nc.vector.tensor_copy(out=pT2[:J, 0:(k - 1) * P + rr],
                                      in_=tp2[:J, 0:(k - 1) * P + rr])

                gt = gpool.tile([P, 6 * D], f32, tag="g", name="gt")
                gv = gt[:rr, 0:k * 3 * D].rearrange("p (k x) -> p k x", x=3 * D)
                hv = htile[:rr, c0 * DS:(c0 + k) * DS].rearrange(
                    "p (k d) -> p k d", d=DS)[:, :, 0:D]
                nc.gpsimd.tensor_copy(out=gv[:, :, 0:D], in_=hv)
                for i, c in enumerate(pair):
                    cps = ps_c2q.tile([P, D], f32, tag="c2q")
                    nc.tensor.matmul(cps[:rr, :], lhsT=pT2[:J, i * P:i * P + rr],
                                     rhs=u_sb[:J, 0:D], start=True, stop=True)
                    nc.tensor.matmul(y_ps[:1, :], lhsT=e_all[:rr, c:c + 1],
                                     rhs=htile[:rr, c * DS:c * DS + D],
                                     start=(c == 0), stop=(c == 6))
                    nc.vector.tensor_scalar_mul(
                        out=gt[:rr, i * 3 * D + D:i * 3 * D + 2 * D],
                        in0=cps[:rr, :], scalar1=rcp2[:rr, i:i + 1])
                nc.gpsimd.tensor_mul(out=gv[:, :, 2 * D:3 * D], in0=hv,
                                     in1=gv[:, :, D:2 * D])
                if k == 2:
                    nc.sync.dma_start(
                        out=g_out[b, t00:t00 + 2 * P, 0:3 * D].rearrange(
                            "(k p) x -> p k x", p=P),
                        in_=gt[:, :].rearrange("p (k x) -> p k x", k=2))
                else:
                    nc.sync.dma_start(out=g_out[b, t00:t00 + rr, 0:3 * D],
                                      in_=gt[:rr, 0:3 * D])

            # ---- batch tail: q2c ----
            S_ps = ps_acc.tile([1, 7], f32, tag="acc")
            nc.tensor.matmul(S_ps[:1, :], lhsT=ones_col[:P, :1], rhs=e_all[:, :],
                             start=True, stop=True)
            Ssum = bsmall.tile([1, 1], f32, tag="Ssum")
            nc.vector.reduce_sum(out=Ssum[:1, :], in_=S_ps[:1, :], axis=AX.X)
            Sinv = bsmall.tile([1, 1], f32, tag="Sinv")
            nc.vector.reciprocal(out=Sinv[:1, :], in_=Ssum[:1, :])
            q2c_sb = bsmall.tile([1, D], f32, tag="q2c")
            nc.vector.tensor_scalar_mul(out=q2c_sb[:1, :], in0=y_ps[:1, 0:D],
                                        scalar1=Sinv[:1, :])
            q2cb_ps = ps_acc.tile([P, D], f32, tag="acc")
            nc.tensor.matmul(q2cb_ps[:, :], lhsT=ones_row[:1, :], rhs=q2c_sb[:1, :],
                             start=True, stop=True)
            q2cb_sb = bsmall.tile([P, D], f32, tag="q2cb")
            nc.scalar.copy(out=q2cb_sb[:, :], in_=q2cb_ps[:, :])

            hq_all = hqpool.tile([P, 7 * D], f32, tag="hq")
            q2cb_b3 = bass.AP(tensor=q2cb_sb.tensor, offset=q2cb_sb.offset,
                              ap=[q2cb_sb.ap[0], [0, 3], q2cb_sb.ap[1]])
            nc.vector.tensor_mul(
                out=hq_all[:, 0:3 * D].rearrange("p (n d) -> p n d", d=D),
                in0=htile[:, 0:3 * DS].rearrange("p (n d) -> p n d", d=DS)[:, :, 0:D],
                in1=q2cb_b3)
            nc.gpsimd.tensor_mul(
                out=hq_all[:, 3 * D:6 * D].rearrange("p (n d) -> p n d", d=D),
                in0=htile[:, 3 * DS:6 * DS].rearrange("p (n d) -> p n d", d=DS)[:, :, 0:D],
                in1=q2cb_b3)
            nc.vector.tensor_mul(out=hq_all[0:T - 6 * P, 6 * D:7 * D],
                                 in0=htile[0:T - 6 * P, 6 * DS:6 * DS + D],
                                 in1=q2cb_sb[0:T - 6 * P, :])
            nc.sync.dma_start(
                out=g_out[b, 0:6 * P, 3 * D:4 * D].rearrange("(n p) d -> p n d", p=P),
                in_=hq_all[:, 0:6 * D].rearrange("p (n d) -> p n d", d=D))
            nc.sync.dma_start(out=g_out[b, 6 * P:T, 3 * D:4 * D],
                              in_=hq_all[0:T - 6 * P, 6 * D:7 * D])

        def run_all():
            for b in range(BC):
                batch_body(b)

        if reps == 1:
            run_all()
        else:
            with tc.For_i(0, reps, 1):
                run_all()

    return nc


def kernel(h, u, w_h, b_h, w_u, b_u, w_hu, b_hu):
    from concourse.bass_utils import run_bass_kernel_spmd

    if "nc" not in _cache:
        nc = _build()
        _split_multi_waits(nc)
        _cache["nc"] = nc
    nc = _cache["nc"]

    h = np.ascontiguousarray(h, dtype=np.float32)
    u = np.ascontiguousarray(u, dtype=np.float32)
    w_h = np.ascontiguousarray(w_h, dtype=np.float32)
    w_u = np.ascontiguousarray(w_u, dtype=np.float32)
    w_hu = np.ascontiguousarray(w_hu, dtype=np.float32)

    core_ids = list(range(NCORES))
    in_maps = []
    for i in core_ids:
        in_maps.append({
            "h": h[i * BC:(i + 1) * BC],
            "u": u[i * BC:(i + 1) * BC],
            "w_h": w_h,
            "w_u": w_u,
            "w_hu": w_hu,
        })
    res = run_bass_kernel_spmd(nc, in_maps, core_ids)
    _cache["last_results"] = res
    return np.concatenate([res.results[i]["g"] for i in core_ids], axis=0)


# revision 37
# speedup vs baseline: 7.6199x; 1.0048x over previous
"""BiDAF attention-flow layer on 8 Trainium2 NeuronCores.

Data-parallel over batch: each core processes B/8 = 8 batches.

Math (per batch b):
  s[t,j] = h[t]·w_h + u[j]·w_u + (h[t]*w_hu)·u[j] + const
  a      = softmax_j(s)            -> only needs  sj = shu + su  (row consts cancel)
  c2q    = a @ u
  bt     = softmax_t(max_j s)      -> needs  m + sh  where m = max_j(sj)
  q2c    = bt @ h
  g      = [h | c2q | h*c2q | h*q2c]

The rank-1 bias terms b_h/b_u/b_hu shift every s[t,j] equally and cancel in
both softmaxes, so they are accepted but unused.

Layout per batch (core-local):
  htile [128, 7*200]   h rows chunked by 128 (chunk c at cols c*200..)
  hT    [100, 2*800]   h transposed (PE transpose pairs, one fused copy)
  s_ps  [128, 51]      cols 0:50 = shu+su (su via K=1 ones-matmul), col 50 = sh
  softmax on free dim; p transposed back (PE) for the c2q matmul;
  y[1,200] = sum_t e_t h_t accumulated in PSUM over chunks; q2c = y/sum(e);
  g cols 0:600 stream out as chunk-pair DMAs, cols 600:800 (needs q2c) as
  two per-batch DMAs. Engines balanced: ACT exp/copies, DVE softmax+scales,
  GPSIMD h-copy + h*c2q, PE matmuls/transposes; all DMAs on the SP HWDGE ring.
"""
import sys

if '/opt/trn_rl_repo' not in sys.path:
    sys.path.insert(0, '/opt/trn_rl_repo')

import numpy as np

B, T, J, D = 64, 800, 50, 200
NCORES = 8
BC = B // NCORES            # batches per core
P = 128
TCHUNKS = [(c * P, min(P, T - c * P)) for c in range((T + P - 1) // P)]
KCHUNKS = [(0, 100), (100, 100)]
NPAD = 256
DS = 201  # htile chunk stride: 200 h cols + a ones column

_cache = {}
F32R = False  # f32r c2q is ~8us faster but 50x less accurate; keep exact


def _split_multi_waits(nc, max_waits=1):
    """This walrus build accepts at most one sync-wait per instruction.
    For any instruction carrying more, move the extra waits onto pure-wait
    EventSemaphore carriers inserted just before it on the same engine —
    the sequencer dispatches in order, so the blocking behavior is
    identical."""
    from concourse import mybir
    import bass_rust
    n = 0
    for f in nc.m.functions:
        for blk in f.blocks:
            insts = blk.instructions
            i = 0
            while i < len(insts):
                inst = insts[i]
                si = inst.sync_info
                if si is not None and len(si.on_wait) > max_waits:
                    waits = list(si.on_wait)
                    keep = waits[-max_waits:]
                    new = []
                    for w in waits[:-max_waits]:
                        d = mybir.InstEventSemaphore(
                            name=f"{inst.name}-sw{n}", ins=[], outs=[])
                        n += 1
                        d.engine = inst.engine
                        d.sync_info = bass_rust.SyncInfo(on_wait=[w], on_update=[])
                        new.append(d)
                    inst.sync_info = bass_rust.SyncInfo(
                        on_wait=keep, on_update=list(si.on_update))
                    for j, d in enumerate(new):
                        insts.insert(i + j, d)
                    i += len(new)
                i += 1
    return n


def _build(reps=1):
    import concourse.bass as bass
    import concourse.tile as tile
    from concourse import mybir, masks
    from contextlib import ExitStack

    f32 = mybir.dt.float32
    f32r = mybir.dt.float32r
    AF = mybir.ActivationFunctionType
    AX = mybir.AxisListType

    nc = bass.Bass()
    h_in = nc.declare_dram_parameter("h", [BC, T, D], f32, isOutput=False)
    u_in = nc.declare_dram_parameter("u", [BC, J, D], f32, isOutput=False)
    wh_in = nc.declare_dram_parameter("w_h", [D], f32, isOutput=False)
    wu_in = nc.declare_dram_parameter("w_u", [D], f32, isOutput=False)
    whu_in = nc.declare_dram_parameter("w_hu", [D], f32, isOutput=False)
    g_out = nc.declare_dram_parameter("g", [BC, T, 4 * D], f32, isOutput=True)

    with tile.TileContext(nc) as tc, ExitStack() as ctx:
        singles = ctx.enter_context(tc.tile_pool(name="singles", bufs=1))
        hpool = ctx.enter_context(tc.tile_pool(name="hpool", bufs=4))
        hTpool = ctx.enter_context(tc.tile_pool(name="hTpool", bufs=3))
        upool = ctx.enter_context(tc.tile_pool(name="upool", bufs=3))
        bsmall = ctx.enter_context(tc.tile_pool(name="bsmall", bufs=4))
        csmall = ctx.enter_context(tc.tile_pool(name="csmall", bufs=6))
        gpool = ctx.enter_context(tc.tile_pool(name="gpool", bufs=8))
        hqpool = ctx.enter_context(tc.tile_pool(name="hqpool", bufs=3))
        ps_s = ctx.enter_context(
            tc.tile_pool(name="ps_s", bufs=2, space=bass.MemorySpace.PSUM))
        ps_tp = ctx.enter_context(
            tc.tile_pool(name="ps_tp", bufs=2, space=bass.MemorySpace.PSUM))
        ps_c2q = ctx.enter_context(
            tc.tile_pool(name="ps_c2q", bufs=2, space=bass.MemorySpace.PSUM))
        ps_acc = ctx.enter_context(
            tc.tile_pool(name="ps_acc", bufs=2, space=bass.MemorySpace.PSUM))

        # ---- once-per-core constants ----
        identity = singles.tile([P, P], f32)
        masks.make_identity(nc, identity[:])
        ones_row = singles.tile([1, P], f32)
        nc.vector.memset(ones_row, 1.0)
        ones_col = singles.tile([P, 1], f32)
        nc.vector.memset(ones_col, 1.0)

        wh0 = singles.tile([96, 1], f32, tag="wh0", name="wh0")
        nc.sync.dma_start(out=wh0[:, :],
                          in_=wh_in[0:96].rearrange("(p one) -> p one", one=1))
        wh1e = singles.tile([105, 1], f32, tag="wh1e", name="wh1e")
        nc.vector.memset(wh1e, 0.0)
        nc.sync.dma_start(out=wh1e[0:104, :],
                          in_=wh_in[96:D].rearrange("(p one) -> p one", one=1))

        def bcast_load(wsrc, name):
            t_ = singles.tile([J, D], f32, tag=name, name=name)
            s_ap = wsrc[:]
            nc.sync.dma_start(out=t_[:, :], in_=bass.AP(
                tensor=s_ap.tensor, offset=s_ap.offset, ap=[[0, J], s_ap.ap[0][:]]))
            return t_

        whu_b = bcast_load(whu_in, "whu_b")
        wu_b = bcast_load(wu_in, "wu_b")

        loaded = {}

        def load_body(b):
            # u first (small; unblocks u-prep), then h
            u_sb = upool.tile([J, D], f32, tag="u", name="u_sb")
            nc.sync.dma_start(out=u_sb[:, 0:D], in_=u_in[b, :, :])
            htile = hpool.tile([P, 7 * DS], f32, tag="h", name="htile")
            nc.gpsimd.memset(
                htile[:, :].rearrange("p (n d) -> p n d", d=DS)[:, :, D:DS], 1.0)
            nmain = 6
            nc.sync.dma_start(
                out=htile[:, 0:nmain * DS].rearrange(
                    "p (n d) -> p n d", d=DS)[:, :, 0:D],
                in_=h_in[b, 0:nmain * P, :].rearrange("(n p) d -> p n d", p=P),
            )
            nc.sync.dma_start(
                out=htile[0:T - nmain * P, nmain * DS:nmain * DS + D],
                in_=h_in[b, nmain * P:T, :],
            )
            loaded[b] = (u_sb, htile)

        def batch_body(b, prefetch=None):
            load_body(b)
            u_sb, htile = loaded.pop(b)

            # ---- u-side prep: u_w = u*w_hu with su appended as col 200;
            # transposing u_w gives the s-matmul rhs (su lands in K-row 104,
            # paired with the ones column embedded in htile) ----
            u_w = upool.tile([J, DS], f32, tag="u_w", name="u_w")
            nc.vector.tensor_mul(out=u_w[:, 0:D], in0=u_sb[:, 0:D],
                                 in1=whu_b[:, :])
            su_t = upool.tile([J, D], f32, tag="su_t", name="su_t")
            nc.vector.tensor_mul(out=su_t[:, :], in0=u_sb[:, 0:D], in1=wu_b[:, :])
            su_col = bsmall.tile([J, 1], f32, tag="su")
            nc.vector.reduce_sum(out=su_col[:, :], in_=su_t[:, :], axis=AX.X)
            nc.gpsimd.tensor_copy(out=u_w[:, D:DS], in_=su_col[:, :])

            rhs_ext = []
            for kc, (d0, kn, whcol) in enumerate(((0, 96, wh0), (96, 105, wh1e))):
                tp = ps_tp.tile([105, P], f32, tag="tp")
                nc.tensor.transpose(tp[:kn, :J], u_w[:J, d0:d0 + kn],
                                    identity[:J, :J])
                re_ = upool.tile([105, J + 1], f32, tag=f"rhs{kc}", name=f"rhs{kc}")
                nc.scalar.copy(out=re_[:kn, 0:J], in_=tp[:kn, :J])
                nc.gpsimd.tensor_copy(out=re_[:kn, J:J + 1], in_=whcol[:kn, :])
                rhs_ext.append(re_)

            # ---- h transpose: hT [105, 2*800]; kc1 row 104 = ones (su).
            # Two chunks' transposes (4 matmuls) share one PSUM tile and
            # drain with a single permuting copy. ----
            hT = hTpool.tile([105, 2 * T], f32, tag="hT")
            for cpair in ((0, 1), (2, 3), (4, 5), (6,)):
                t0 = cpair[0] * P
                ck = len(cpair)
                rows = TCHUNKS[cpair[-1]][1]
                tp = ps_tp.tile([105, 4 * P], f32, tag="tp")
                for ci, c in enumerate(cpair):
                    for kc, d0 in enumerate((0, 96)):
                        nc.tensor.matmul(
                            tp[:105, (2 * ci + kc) * P:(2 * ci + kc) * P + rows],
                            lhsT=htile[:rows, c * DS + d0:c * DS + d0 + 105],
                            rhs=identity[:rows, :rows], is_transpose=True,
                            skip_group_check=True)
                # src [p, (c k x)] -> dest hT [p, (k t)] with t = c*128 + x
                if ck == 2:
                    nc.scalar.copy(
                        out=hT[:105, :].rearrange("p (k t) -> p k t", k=2)
                        [:, :, t0:t0 + 2 * P].rearrange(
                            "p k (c x) -> p k c x", x=P),
                        in_=tp[:105, :].rearrange(
                            "p (c k x) -> p k c x", k=2, x=P))
                else:
                    nc.scalar.copy(
                        out=hT[:105, :].rearrange("p (k t) -> p k t", k=2)
                        [:, :, t0:t0 + rows],
                        in_=tp[:105, 0:2 * P].rearrange(
                            "p (k x) -> p k x", k=2)[:, :, :rows])

            e_all = bsmall.tile([P, 7], f32, tag="e_all")
            nc.gpsimd.memset(e_all, 0.0)
            y_ps = ps_acc.tile([1, D], f32, tag="acc", name="y_ps")

            # ---- main chunk loop: chunks processed in pairs so the
            # softmax reductions/exp/copies run as one [128, 2, 50] op each ----
            assert not F32R
            for pair in ((0, 1), (2, 3), (4, 5), (6,)):
                c0 = pair[0]
                k = len(pair)
                t00 = c0 * P
                rr = TCHUNKS[pair[-1]][1]   # 128 for full pairs, 32 for (6,)
                s2 = ps_s.tile([P, 2 * (J + 1)], f32, tag="s", name="s2")
                for i, c in enumerate(pair):
                    t0, rows = TCHUNKS[c]
                    so = i * (J + 1)
                    nc.tensor.matmul(s2[:rows, so:so + J + 1],
                                     lhsT=hT[0:96, t0:t0 + rows],
                                     rhs=rhs_ext[0][:96, :], start=True, stop=False)
                    nc.tensor.matmul(s2[:rows, so:so + J + 1],
                                     lhsT=hT[0:105, T + t0:T + t0 + rows],
                                     rhs=rhs_ext[1][:105, :], start=False, stop=True)

                s2v = s2[:rr, :].rearrange("p (k j) -> p k j", j=J + 1)[:, 0:k, :]
                m2 = csmall.tile([P, 2], f32, tag="m")
                nc.vector.reduce_max(out=m2[:rr, 0:k], in_=s2v[:, :, 0:J], axis=AX.X)
                msh2 = csmall.tile([P, 2], f32, tag="msh")
                nc.vector.tensor_add(
                    out=msh2[:rr, 0:k].rearrange("p (k one) -> p k one", one=1),
                    in0=m2[:rr, 0:k].rearrange("p (k one) -> p k one", one=1),
                    in1=s2v[:, :, J:J + 1])
                nc.scalar.activation(out=e_all[:rr, c0:c0 + k], in_=msh2[:rr, 0:k],
                                     func=AF.Exp)
                p2 = csmall.tile([P, 2 * J], f32, tag="p", name="p2")
                nc.scalar.activation(
                    out=p2[:rr, 0:k * J].rearrange("p (k j) -> p k j", j=J),
                    in_=s2v[:, :, 0:J], func=AF.Exp)
                rs2 = csmall.tile([P, 2], f32, tag="rsum")
                nc.vector.reduce_sum(
                    out=rs2[:rr, 0:k],
                    in_=p2[:rr, 0:k * J].rearrange("p (k j) -> p k j", j=J),
                    axis=AX.X)
                rcp2 = csmall.tile([P, 2], f32, tag="rcp")
                nc.vector.reciprocal(out=rcp2[:rr, 0:k], in_=rs2[:rr, 0:k])

                tp2 = ps_tp.tile([100, 2 * P], f32, tag="tp")
                for i, c in enumerate(pair):
                    nc.tensor.matmul(tp2[:J, i * P:i * P + rr],
                                     lhsT=p2[:rr, i * J:(i + 1) * J],
                                     rhs=identity[:rr, :rr], is_transpose=True,
                                     skip_group_check=True)
                pT2 = csmall.tile([J, 2 * P], f32, tag="pT", name="pT2")
                nc.vector.tensor_copy(out=pT2[:J, 0:(k - 1) * P + rr],
                                      in_=tp2[:J, 0:(k - 1) * P + rr])

                gt = gpool.tile([P, 6 * D], f32, tag="g", name="gt")
                gv = gt[:rr, 0:k * 3 * D].rearrange("p (k x) -> p k x", x=3 * D)
                hv = htile[:rr, c0 * DS:(c0 + k) * DS].rearrange(
                    "p (k d) -> p k d", d=DS)[:, :, 0:D]
                nc.gpsimd.tensor_copy(out=gv[:, :, 0:D], in_=hv)
                for i, c in enumerate(pair):
                    cps = ps_c2q.tile([P, D], f32, tag="c2q")
                    nc.tensor.matmul(cps[:rr, :], lhsT=pT2[:J, i * P:i * P + rr],
                                     rhs=u_sb[:J, 0:D], start=True, stop=True)
                    nc.tensor.matmul(y_ps[:1, :], lhsT=e_all[:rr, c:c + 1],
                                     rhs=htile[:rr, c * DS:c * DS + D],
                                     start=(c == 0), stop=(c == 6))
                    nc.vector.tensor_scalar_mul(
                        out=gt[:rr, i * 3 * D + D:i * 3 * D + 2 * D],
                        in0=cps[:rr, :], scalar1=rcp2[:rr, i:i + 1])
                nc.gpsimd.tensor_mul(out=gv[:, :, 2 * D:3 * D], in0=hv,
                                     in1=gv[:, :, D:2 * D])
                if k == 2:
                    nc.sync.dma_start(
                        out=g_out[b, t00:t00 + 2 * P, 0:3 * D].rearrange(
                            "(k p) x -> p k x", p=P),
                        in_=gt[:, :].rearrange("p (k x) -> p k x", k=2))
                else:
                    nc.sync.dma_start(out=g_out[b, t00:t00 + rr, 0:3 * D],
                                      in_=gt[:rr, 0:3 * D])

            # ---- batch tail: q2c ----
            S_ps = ps_acc.tile([1, 7], f32, tag="acc")
            nc.tensor.matmul(S_ps[:1, :], lhsT=ones_col[:P, :1], rhs=e_all[:, :],
                             start=True, stop=True)
            Ssum = bsmall.tile([1, 1], f32, tag="Ssum")
            nc.vector.reduce_sum(out=Ssum[:1, :], in_=S_ps[:1, :], axis=AX.X)
            Sinv = bsmall.tile([1, 1], f32, tag="Sinv")
            nc.vector.reciprocal(out=Sinv[:1, :], in_=Ssum[:1, :])
            q2c_sb = bsmall.tile([1, D], f32, tag="q2c")
            nc.vector.tensor_scalar_mul(out=q2c_sb[:1, :], in0=y_ps[:1, 0:D],
                                        scalar1=Sinv[:1, :])
            q2cb_ps = ps_acc.tile([P, D], f32, tag="acc")
            nc.tensor.matmul(q2cb_ps[:, :], lhsT=ones_row[:1, :], rhs=q2c_sb[:1, :],
                             start=True, stop=True)
            q2cb_sb = bsmall.tile([P, D], f32, tag="q2cb")
            nc.scalar.copy(out=q2cb_sb[:, :], in_=q2cb_ps[:, :])

            hq_all = hqpool.tile([P, 7 * D], f32, tag="hq")
            q2cb_b3 = bass.AP(tensor=q2cb_sb.tensor, offset=q2cb_sb.offset,
                              ap=[q2cb_sb.ap[0], [0, 3], q2cb_sb.ap[1]])
            nc.vector.tensor_mul(
                out=hq_all[:, 0:3 * D].rearrange("p (n d) -> p n d", d=D),
                in0=htile[:, 0:3 * DS].rearrange("p (n d) -> p n d", d=DS)[:, :, 0:D],
                in1=q2cb_b3)
            nc.gpsimd.tensor_mul(
                out=hq_all[:, 3 * D:6 * D].rearrange("p (n d) -> p n d", d=D),
                in0=htile[:, 3 * DS:6 * DS].rearrange("p (n d) -> p n d", d=DS)[:, :, 0:D],
                in1=q2cb_b3)
            nc.vector.tensor_mul(out=hq_all[0:T - 6 * P, 6 * D:7 * D],
                                 in0=htile[0:T - 6 * P, 6 * DS:6 * DS + D],
                                 in1=q2cb_sb[0:T - 6 * P, :])
            nc.sync.dma_start(
                out=g_out[b, 0:6 * P, 3 * D:4 * D].rearrange("(n p) d -> p n d", p=P),
                in_=hq_all[:, 0:6 * D].rearrange("p (n d) -> p n d", d=D))
            nc.sync.dma_start(out=g_out[b, 6 * P:T, 3 * D:4 * D],
                              in_=hq_all[0:T - 6 * P, 6 * D:7 * D])

        def run_all():
            for b in range(BC):
                batch_body(b)

        if reps == 1:
            run_all()
        else:
            with tc.For_i(0, reps, 1):
                run_all()

    return nc


def kernel(h, u, w_h, b_h, w_u, b_u, w_hu, b_hu):
    from concourse.bass_utils import run_bass_kernel_spmd

    if "nc" not in _cache:
        nc = _build()
        _split_multi_waits(nc)
        _cache["nc"] = nc
    nc = _cache["nc"]

    h = np.ascontiguousarray(h, dtype=np.float32)
    u = np.ascontiguousarray(u, dtype=np.float32)
    w_h = np.ascontiguousarray(w_h, dtype=np.float32)
    w_u = np.ascontiguousarray(w_u, dtype=np.float32)
    w_hu = np.ascontiguousarray(w_hu, dtype=np.float32)

    core_ids = list(range(NCORES))
    in_maps = []
    for i in core_ids:
        in_maps.append({
            "h": h[i * BC:(i + 1) * BC],
            "u": u[i * BC:(i + 1) * BC],
            "w_h": w_h,
            "w_u": w_u,
            "w_hu": w_hu,
        })
    res = run_bass_kernel_spmd(nc, in_maps, core_ids)
    _cache["last_results"] = res
    return np.concatenate([res.results[i]["g"] for i in core_ids], axis=0)
